# revision 68
# baseline (speedup 1.0000x reference)
"""GQA attention (B=2,T=2048,D=2048,H=16,KV=4,HD=128, causal+RoPE) on 8 trn2 cores.

Sharding: 4-way head tensor-parallel x 2-way batch data-parallel.
Core c: batch b=c//4, TP shard s=c%4 -> q heads [4s..4s+3], kv head s.

Transpose-free design (v1): scores are computed directly in kv-major layout
per 128-token kv block:  ST[kv,q] = kT_block^T @ qT_stripe  (PE), so
exp(ST) written to SBUF *is* the P^T operand needed by the PV matmul
O^T[hd,q] = V_block^T @ P^T.  The softmax denominator l[q] = colsum(P^T)
comes from a ones-vector matmul ([128,1] lhsT), its reciprocal is
broadcast to all partitions with a K=1 outer-product matmul, and the
normalization is fused into the PSUM->SBUF move of O^T on the DVE.
Causal masking is multiplicative post-exp (binary bf16 tiles) on the 4
diagonal blocks of each 512-wide q stripe.  No PE transposes anywhere,
so the PE stream is dense back-to-back matmuls and the HAM clock gate
stays at 2.4 GHz.
"""

import math
import os
import numpy as np

try:
    import concourse.bass as bass
except ImportError:  # pragma: no cover
    import sys

    sys.path.insert(0, "/opt/trn_rl_repo")
    import concourse.bass as bass

import concourse.mybir as mybir
import concourse.bacc as bacc
from concourse import bass_utils
from concourse.tile import TileContext
from contextlib import ExitStack
from ml_dtypes import bfloat16, float8_e4m3

B, T, D = 2, 2048, 2048
H, KV, HD = 16, 4, 128
TP = 4  # head-TP ways
NH = H // TP  # q heads per core = 4
NKB = D // 128  # 16 contraction blocks
NTC = T // 512  # 4 token chunks / q stripes
NTB = T // 128  # 16 token blocks
SCALE = 1.0 / math.sqrt(HD)
F32 = mybir.dt.float32
BF16 = mybir.dt.bfloat16
FP8 = mybir.dt.float8e4
DR = mybir.MatmulPerfMode.DoubleRow
EXP = mybir.ActivationFunctionType.Exp

_program = None
_last_results = None
last_exec_time_ns = None
DEFER = bool(int(os.environ.get("KERNEL_DEFER", "1")))  # filler interleave on/off


def _build_program():
    global _program
    if _program is not None:
        return _program

    nc = bacc.Bacc(
        "TRN2",
        target_bir_lowering=False,
        debug=False,
        enable_asserts=False,
        num_devices=8,
    )
    # host-packed layouts: [128 partitions, ...] with j = D/128 contraction blocks
    xp_d = nc.dram_tensor("xp", [128, NKB, 512], BF16, kind="ExternalInput").ap()
    xp8_d = nc.dram_tensor("xp8", [128, 3, NKB, 512], FP8, kind="ExternalInput").ap()
    wq_d = nc.dram_tensor("Wq", [128, NKB, NH * 128], BF16, kind="ExternalInput").ap()
    wk_d = nc.dram_tensor("Wk", [128, NKB, 128], BF16, kind="ExternalInput").ap()
    wv_d = nc.dram_tensor("Wv", [128, NKB, 128], BF16, kind="ExternalInput").ap()
    wq8_d = nc.dram_tensor("Wq8", [128, NKB, NH * 128], FP8, kind="ExternalInput").ap()
    wk8_d = nc.dram_tensor("Wk8", [128, NKB, 128], FP8, kind="ExternalInput").ap()
    wv8_d = nc.dram_tensor("Wv8", [128, NKB, 128], FP8, kind="ExternalInput").ap()
    wo_d = nc.dram_tensor("Wo", [128, NH, D], BF16, kind="ExternalInput").ap()
    cq_d = nc.dram_tensor("cosq", [128, T], BF16, kind="ExternalInput").ap()
    sq_d = nc.dram_tensor("sinq", [128, T], BF16, kind="ExternalInput").ap()
    ck_d = nc.dram_tensor("cosk", [128, T], BF16, kind="ExternalInput").ap()
    sk_d = nc.dram_tensor("sink", [128, T], BF16, kind="ExternalInput").ap()
    mk_d = nc.dram_tensor("maskv", [128, 4, 512], BF16, kind="ExternalInput").ap()
    id_d = nc.dram_tensor("ident", [128, 128], BF16, kind="ExternalInput").ap()
    y_d = nc.dram_tensor("y", [T, D], BF16, kind="ExternalOutput").ap()

    with TileContext(nc) as tc, ExitStack() as ctx:
        big = ctx.enter_context(tc.tile_pool(name="big", bufs=1))
        xpool = ctx.enter_context(tc.tile_pool(name="xpool", bufs=2))
        ps = ctx.enter_context(tc.tile_pool(name="ps", bufs=2, space="PSUM"))
        ps_l = ctx.enter_context(tc.tile_pool(name="ps_l", bufs=2, space="PSUM"))
        ps_y = ctx.enter_context(tc.tile_pool(name="ps_y", bufs=2, space="PSUM"))
        rtmp = ctx.enter_context(tc.tile_pool(name="rtmp", bufs=3))
        vtpool = ctx.enter_context(tc.tile_pool(name="vtpool", bufs=2))
        ptpool = ctx.enter_context(tc.tile_pool(name="ptpool", bufs=2))
        otpool = ctx.enter_context(tc.tile_pool(name="otpool", bufs=2))
        rlpool = ctx.enter_context(tc.tile_pool(name="rlpool", bufs=2))
        bcpool = ctx.enter_context(tc.tile_pool(name="bcpool", bufs=2))
        ypool = ctx.enter_context(tc.tile_pool(name="ypool", bufs=2))

        wq = big.tile([128, NKB, NH * 128], BF16, tag="wq")  # loaded in 4 pieces
        wk = big.tile([128, NKB, 128], BF16, tag="wk")
        wv = big.tile([128, NKB, 128], BF16, tag="wv")
        wq8 = big.tile([128, NKB, NH * 128], FP8, tag="wq8")
        wk8 = big.tile([128, NKB, 128], FP8, tag="wk8")
        wv8 = big.tile([128, NKB, 128], FP8, tag="wv8")
        wo = big.tile([128, NH, D], BF16, tag="wo")
        cq = big.tile([128, T], BF16, tag="cq")
        sq = big.tile([128, T], BF16, tag="sq")
        ck = big.tile([128, T], BF16, tag="ck")
        sk = big.tile([128, T], BF16, tag="sk")
        maskv = big.tile([128, 4, 512], BF16, tag="maskv")
        ident = big.tile([128, 128], BF16, tag="ident")
        onesc = big.tile([128, 1], BF16, tag="onesc")
        onesr = big.tile([1, 128], BF16, tag="onesr")
        warm = big.tile([128, 512], BF16, tag="warm")
        wsink = big.tile([128, 16], F32, tag="wsink")
        qT = big.tile([128, NH, T], BF16, tag="qT")
        kT = big.tile([128, T], BF16, tag="kT")
        V = big.tile([128, NTB, 128], BF16, tag="V")

        # ---- PE warmup: ~6us of dummy matmuls during the initial DMA wait
        # gets the HAM clock gate to 2.4 GHz before the first real matmul.
        nc.vector.memset(warm[:], 0.125)
        nc.vector.memset(onesc[:], 1.0)
        nc.vector.memset(onesr[:], 1.0)
        wp = ps.tile([128, 2, 512], F32, tag="st", name="pst")[:, 0, :]
        for i in range(4):
            nc.tensor.matmul(
                wp[:], lhsT=warm[:, :128], rhs=warm[:], start=(i == 0), stop=(i == 3)
            )
        nc.vector.tensor_copy(wsink[:], wp[:, :16])

        # ---- loads (ordered so early compute unblocks fast) ----
        xcs = {}

        def fetch_x(c):
            if c == 0:
                t = xpool.tile([128, NKB, 512], BF16, tag="xc", bufs=1)
                nc.sync.dma_start(out=t[:], in_=xp_d[:])
            else:
                t = xpool.tile([128, NKB, 512], FP8, tag="xc8")
                nc.sync.dma_start(out=t[:], in_=xp8_d[:, c - 1])
            xcs[c] = t

        # fp8 essentials first: chunk 1 (fp8) is the first compute and needs
        # only ~2MB to start; the 5MB bf16 set for chunk 0 streams in behind
        # ~22us of fp8 chunk-1/2 compute.
        # startup loads split across BOTH hardware DGE queues (sync + scalar)
        # so the chunk-1 gate (~2MB of fp8) transfers in parallel streams.
        x1t = xpool.tile([128, NKB, 512], FP8, tag="xc8", name="x1t")
        nc.sync.dma_start(out=wk8[:], in_=wk8_d[:])
        nc.scalar.dma_start(out=x1t[:, 8:], in_=xp8_d[:, 0, 8:])
        nc.sync.dma_start(out=x1t[:, :8], in_=xp8_d[:, 0, :8])
        xcs[1] = x1t
        nc.scalar.dma_start(out=ck[:], in_=ck_d[:])
        nc.sync.dma_start(out=sk[:], in_=sk_d[:])
        nc.scalar.dma_start(out=wq8[:], in_=wq8_d[:])
        nc.sync.dma_start(out=cq[:], in_=cq_d[:])
        nc.scalar.dma_start(out=sq[:], in_=sq_d[:])
        nc.sync.dma_start(out=wv8[:], in_=wv8_d[:])
        nc.scalar.dma_start(out=ident[:], in_=id_d[:])
        fetch_x(2)
        nc.scalar.dma_start(out=wk[:], in_=wk_d[:])
        fetch_x(0)
        for jp in range(4):
            eng = nc.scalar if jp % 2 else nc.sync
            eng.dma_start(
                out=wq[:, 4 * jp : 4 * (jp + 1), :], in_=wq_d[:, 4 * jp : 4 * (jp + 1), :]
            )
        nc.scalar.dma_start(out=wv[:], in_=wv_d[:])
        nc.sync.dma_start(out=maskv[:], in_=mk_d[:])
        nc.sync.dma_start(out=wo[:], in_=wo_d[:])

        # ---- projections with fused RoPE, chunk-major over tokens ----
        # The PSUM->SBUF staging copy (ACT) frees the psum slot right away so
        # a late rope (waiting on cos/sin DMAs) never stalls the PE via pool
        # WAR; rope then runs from SBUF at its leisure.
        def rope(pst, cos_sb, sin_sb, dst, sl, gadd=False):
            # 3-way engine split (ops are free-dim-bound, ~0.45-0.7us per
            # 512-wide op regardless of partitions): ACT lifts the
            # half-swapped copy out of PSUM, DVE does the cos mul (PSUM
            # read) + final add, gpsimd the sin mul (SBUF only).
            pqs = rtmp.tile([128, 512], BF16, tag="pqs", bufs=4)
            nc.scalar.copy(pqs[0:64, :], pst[64:128, :])
            nc.scalar.copy(pqs[64:128, :], pst[0:64, :])
            t1 = rtmp.tile([128, 512], BF16, tag="t1")
            nc.vector.tensor_mul(t1[:], pst[:], cos_sb[:, sl])
            t2 = rtmp.tile([128, 512], BF16, tag="t2")
            nc.gpsimd.tensor_mul(t2[:], pqs[:], sin_sb[:, sl])
            if gadd:
                nc.gpsimd.tensor_add(dst, t1[:], t2[:])
            else:
                nc.vector.tensor_add(dst, t1[:], t2[:])

        # projection matmuls: chunk 0 runs bf16 (16 K=128 matmuls); chunks
        # 1-3 run fp8 DoubleRow (8 K=256 matmuls over adjacent j pairs) --
        # early tokens see little softmax averaging so they keep bf16.
        def proj(pst, wbf, wf8, xc, c, cs=slice(None)):
            if c == 0:
                for j in range(NKB):
                    nc.tensor.matmul(
                        pst[:],
                        lhsT=wbf[:, j, cs],
                        rhs=xc[:, j, :],
                        start=(j == 0),
                        stop=(j == NKB - 1),
                    )
            else:
                for u in range(NKB // 2):
                    nc.tensor.matmul(
                        pst[:],
                        lhsT=wf8[:, 2 * u : 2 * u + 2, cs],
                        rhs=xc[:, 2 * u : 2 * u + 2, :],
                        start=(u == 0),
                        stop=(u == NKB // 2 - 1),
                        perf_mode=DR,
                    )

        # Each chunk's projection work as a list of closures ("units"):
        # chunks 0-2 are emitted inline; chunk 3's units go to the filler
        # queue and are interleaved into stripe-0 attention so the PE never
        # idles on the exp latency there (that idle used to re-throttle HAM).
        def chunk_units(c, fetch_next=None):
            sl = slice(c * 512, (c + 1) * 512)
            xc = xcs.pop(c)
            units = []

            gadd = c == NTC - 1  # filler-chunk rope adds ride on gpsimd

            def kt_unit():
                pst = ps.tile([128, 2, 512], F32, tag="st", name="pst")[:, 0, :]
                proj(pst, wk, wk8, xc, c)
                rope(pst, ck, sk, kT[:, sl], sl, gadd)

            units.append(kt_unit)
            for h in range(NH):

                def qt_unit(h=h):
                    pst = ps.tile([128, 2, 512], F32, tag="st", name="pst")[:, 0, :]
                    proj(pst, wq, wq8, xc, c, slice(h * 128, (h + 1) * 128))
                    rope(pst, cq, sq, qT[:, h, sl], sl, gadd)

                units.append(qt_unit)

            def v_unit():
                # V^T chunk (hd on partitions), then 128x128 PE transposes
                # into the tokens-major V needed by the PV matmul.
                pst = ps.tile([128, 2, 512], F32, tag="st", name="pst")[:, 0, :]
                proj(pst, wv, wv8, xc, c)
                vtc = vtpool.tile([128, 512], BF16, tag="vt")
                nc.scalar.copy(vtc[:], pst[:])
                for tb in range(4):
                    tp = ps.tile([128, 2, 512], BF16, tag="st", name="tp")[:, 0, :128]
                    nc.tensor.transpose(
                        tp[:], vtc[:, tb * 128 : (tb + 1) * 128], ident[:]
                    )
                    nc.vector.tensor_copy(V[:, c * 4 + tb, :], tp[:])
                if fetch_next is not None:
                    fetch_x(fetch_next)

            units.append(v_unit)
            return units

        filler = []

        def feed(n=1):
            for _ in range(n):
                if not filler:
                    return
                filler.pop(0)()

        # chunk order 1, 2, 0, 3: the two fp8 chunks run while chunk 0's
        # bf16 weights/x stream in.  Chunk 3 plus the tail of chunk 2 (qT
        # h2/h3 + V, first needed by stripe 2) fill stripe-0 attention.
        c2_units = chunk_units(2, None)
        for u in chunk_units(1, 3):
            u()
        for u in c2_units[:3]:
            u()
        for u in chunk_units(0, None):
            u()
        if DEFER:
            filler.extend(c2_units[3:])
            filler.extend(chunk_units(NTC - 1))
        else:
            for u in c2_units[3:]:
                u()
            for u in chunk_units(NTC - 1):
                u()

        # ---- attention + output projection, per q stripe ----
        LN = mybir.ActivationFunctionType.Ln
        for s in range(NTC):
            qsl = slice(s * 512, (s + 1) * 512)
            nb = 4 * (s + 1)
            OTs = otpool.tile([128, NH, 512], BF16, tag="OT")

            # finalize head h: reciprocal on the narrow [1,512] l row first,
            # broadcast 1/l across partitions (K=1 outer product on PE),
            # stage to SBUF on ACT, then one DVE mul normalizes O^T.
            def finalize(h, lps, op, ot):
                # reciprocal straight from the [1,512] l-row in PSUM (frees
                # the ps_l slot), tiny bf16 cast, PE broadcast, ACT stage,
                # one DVE mul to normalize O^T.
                rl32 = rlpool.tile([1, 512], F32, tag="rl32")
                nc.vector.reciprocal_approx_fast(rl32[:], lps[:])
                rlb = rlpool.tile([1, 512], BF16, tag="rlb")
                nc.vector.tensor_copy(rlb[:], rl32[:])
                bc = ps.tile([128, 2, 512], F32, tag="st", name="pst")[:, 0, :]
                nc.tensor.matmul(bc[:], lhsT=onesr[:], rhs=rlb[:], start=True, stop=True)
                bcs = bcpool.tile([128, 512], BF16, tag="bcs")
                nc.scalar.copy(bcs[:], bc[:])
                nc.vector.tensor_mul(ot[:, h, :], op[:], bcs[:])

            pending = None
            for h in range(NH):
                PT = ptpool.tile([128, NTB, 512], BF16, tag="PT")
                PTacc = ptpool.tile([128, 2, 512], BF16, tag="PTacc")
                lp = ps_l.tile([1, 512], F32, tag="lp")
                op = ps_y.tile([128, 512], F32, tag="acc", name="op")

                # pair g covering kv blocks (2g, 2g+1); the second diagonal
                # pair only touches q columns [256:512] (the rest is masked),
                # so S/exp/mask run on the narrowed pair window, and PV on a
                # per-block window.  The softmax denominator is a pair-wide
                # bf16 running sum on the DVE, reduced over partitions with
                # two accumulating ones-matmuls per head.
                npairs = nb // 2

                def wlo(g):
                    return 256 if 2 * g == 4 * s + 2 else 0

                def blo(b):
                    r = b - 4 * s
                    return 128 * r if r > 0 else 0

                def psum_acc2(g):
                    # one [128, 2, W] DVE op per kv-block pair; the narrower
                    # block's extra columns are mask-zeroed so they add 0
                    lo = blo(2 * g)
                    if g == 0:
                        nc.vector.tensor_copy(PTacc[:], PT[:, 0:2, :])
                    else:
                        nc.vector.tensor_add(
                            PTacc[:, :, lo:], PTacc[:, :, lo:], PT[:, 2 * g : 2 * g + 2, lo:]
                        )

                def pvmm(b):
                    lo = blo(b)
                    nc.tensor.matmul(
                        op[:, lo:],
                        lhsT=V[:, b, :],
                        rhs=PT[:, b, lo:],
                        start=(b == 0),
                        stop=(b == nb - 1),
                        skip_group_check=True,
                    )

                def lpvq(g0, g1):
                    for g in (g0, g1):
                        pvmm(2 * g)
                        pvmm(2 * g + 1)

                for g in range(npairs):
                    lo = wlo(g)
                    stg = ps.tile([128, 2, 512], F32, tag="st")
                    for u in (0, 1):
                        b = 2 * g + u
                        nc.tensor.matmul(
                            stg[:, u, lo:],
                            lhsT=kT[:, b * 128 : (b + 1) * 128],
                            rhs=qT[:, h, s * 512 + lo : (s + 1) * 512],
                            start=True,
                            stop=True,
                        )
                    nc.scalar.activation(
                        PT[:, 2 * g : 2 * g + 2, lo:], stg[:, :, lo:], EXP
                    )
                    if 2 * g >= 4 * s:
                        # mask rows r, r+1 are all-ones beyond lo+256, so the
                        # multiply only needs the 256-wide diagonal window
                        r = 2 * g - 4 * s
                        nc.vector.tensor_mul(
                            PT[:, 2 * g : 2 * g + 2, lo : lo + 256],
                            PT[:, 2 * g : 2 * g + 2, lo : lo + 256],
                            maskv[:, r : r + 2, lo : lo + 256],
                        )
                    if g >= 3 and g % 2 == 1:
                        lpvq(g - 3, g - 2)
                    # l-accumulation emitted after the PV issue point: the
                    # counter-based DVE semaphores make PV wait on every DVE
                    # op emitted before its mask-mul, so keep PTacc behind it
                    psum_acc2(g)
                    if g == 0 and pending is not None and len(pending) == 4:
                        ph, pacc, plp, pop = pending
                        nc.tensor.matmul(
                            plp[:], lhsT=onesc[:], rhs=pacc[:, 0, :],
                            start=True, stop=False,
                        )
                        nc.tensor.matmul(
                            plp[:], lhsT=onesc[:], rhs=pacc[:, 1, :],
                            start=False, stop=True,
                        )
                        pending = (ph, plp, pop)
                    if g == 1 and pending is not None:
                        finalize(*pending, OTs)
                        pending = None
                    if g >= 2:
                        feed(1)
                feed(3 if s == 0 else 1)
                lpvq(npairs - 2, npairs - 1)
                pending = (h, PTacc, lp, op)

            # Wo: y[q, :] = sum_h O_h[q, :] @ Wo_h, deferred as filler units
            # fed into the NEXT stripe's attention stream (so exp latency and
            # head boundaries there hide behind dense Wo matmuls).  Unit A
            # finalizes the last head (l-matmul, broadcast, normalize); each
            # (qb, dc) unit is a 4-matmul accumulation into a fast-rotating
            # "st"-pool bank -- never ps_y/ps_l, whose slots pace the next
            # stripe's PV/l chain (a WAR wait there would deadlock the PE
            # FIFO against not-yet-emitted instructions).
            def wo_units(s, OTs, pend):
                ph, pacc, plp, pop = pend
                units = []

                def unit_a():
                    nc.tensor.matmul(
                        plp[:], lhsT=onesc[:], rhs=pacc[:, 0, :],
                        start=True, stop=False,
                    )
                    nc.tensor.matmul(
                        plp[:], lhsT=onesc[:], rhs=pacc[:, 1, :],
                        start=False, stop=True,
                    )
                    finalize(ph, plp, pop, OTs)

                units.append(unit_a)
                ysbs = {}

                def unit_qd(qb, dc):
                    def run():
                        if dc == 0:
                            ysbs[qb] = ypool.tile([128, D], BF16, tag="y", name="ysb")
                        ysb = ysbs[qb]
                        yp = ps.tile([128, 2, 512], F32, tag="st", name="yp")[:, 0, :]
                        for h in range(NH):
                            nc.tensor.matmul(
                                yp[:],
                                lhsT=OTs[:, h, qb * 128 : (qb + 1) * 128],
                                rhs=wo[:, h, dc * 512 : (dc + 1) * 512],
                                start=(h == 0),
                                stop=(h == NH - 1),
                                skip_group_check=True,
                            )
                        # engine per destination stripe: Wo(0) runs inside
                        # stripe 1 (DVE-paced) -> all-ACT; Wo(2) runs inside
                        # stripe 3 (exp/ACT-paced) -> all-DVE; others split.
                        if s == 0:
                            dve = False
                        elif s == 2:
                            dve = True
                        else:
                            dve = dc % 2 == 0
                        if dve:
                            nc.vector.tensor_copy(
                                ysb[:, dc * 512 : (dc + 1) * 512], yp[:]
                            )
                        else:
                            nc.scalar.copy(ysb[:, dc * 512 : (dc + 1) * 512], yp[:])
                        if dc == 3:
                            nc.sync.dma_start(
                                out=y_d[
                                    s * 512 + qb * 128 : s * 512 + (qb + 1) * 128, :
                                ],
                                in_=ysb[:],
                            )

                    return run

                for qb in range(4):
                    for dc in range(4):
                        units.append(unit_qd(qb, dc))
                return units

            if DEFER:
                filler.extend(wo_units(s, OTs, pending))
            else:
                for u in wo_units(s, OTs, pending):
                    u()
            pending = None

        while filler:
            feed(1)

    nc.compile()
    _program = nc
    return nc


def _host_prep(x, Wq, Wk, Wv, Wo):
    x = np.asarray(x, dtype=np.float32)
    Wq = np.asarray(Wq, dtype=np.float32)
    Wk = np.asarray(Wk, dtype=np.float32)
    Wv = np.asarray(Wv, dtype=np.float32)
    Wo = np.asarray(Wo, dtype=np.float32)

    # RoPE even/odd gather folded into weight column permutation (per head)
    perm128 = np.r_[np.arange(0, 128, 2), np.arange(1, 128, 2)]
    permq = np.concatenate([hb * 128 + perm128 for hb in range(H)])
    permk = np.concatenate([hb * 128 + perm128 for hb in range(KV)])
    Wq_p = Wq[:, permq]
    Wk_p = Wk[:, permk]

    pos = np.arange(T, dtype=np.float64)
    inv_freq = 1.0 / (10000.0 ** (np.arange(0, HD, 2, dtype=np.float64) / HD))
    ang = np.einsum("t,f->tf", pos, inv_freq)  # [T, 64]
    cos = np.cos(ang).T.astype(np.float32)  # [64, T]
    sin = np.sin(ang).T.astype(np.float32)
    cosk_f = np.concatenate([cos, cos], axis=0)  # [128, T]
    sink_f = np.concatenate([-sin, sin], axis=0)
    cosq = (cosk_f * SCALE).astype(bfloat16)
    sinq = (sink_f * SCALE).astype(bfloat16)
    cosk = cosk_f.astype(bfloat16)
    sink = sink_f.astype(bfloat16)

    # binary causal masks for the 4 diagonal blocks of a 512-wide q stripe:
    # keep (kv_l <= q_l - 128*r) for relative kv block r
    kv_l = np.arange(128)[:, None]
    q_l = np.arange(512)[None, :]
    maskv = np.ascontiguousarray(
        np.stack([(kv_l <= q_l - 128 * r) for r in range(4)], axis=1)
    ).astype(bfloat16)  # [128, 4, 512]
    ident = np.eye(128, dtype=bfloat16)

    def pack_pj(w, cols, dt=bfloat16):
        # [D, cols] -> [128, NKB, cols] with [p, j, c] = w[j*128+p, c]
        return np.ascontiguousarray(
            w.reshape(NKB, 128, cols).transpose(1, 0, 2)
        ).astype(dt)

    in_maps = []
    for c in range(8):
        b, s = c // 4, c % 4
        xb = np.ascontiguousarray(x[b].T)  # [D, T]
        wo_sh = Wo[s * 512 : (s + 1) * 512, :]  # [512, D]
        in_maps.append(
            {
                "xp": pack_pj(xb[:, :512], 512),
                "xp8": np.ascontiguousarray(
                    np.stack(
                        [
                            pack_pj(xb[:, 512 * cc : 512 * (cc + 1)], 512, float8_e4m3)
                            for cc in range(1, 4)
                        ],
                        axis=1,
                    )
                ),
                "Wq": pack_pj(Wq_p[:, s * 512 : (s + 1) * 512], NH * 128),
                "Wk": pack_pj(Wk_p[:, s * 128 : (s + 1) * 128], 128),
                "Wv": pack_pj(Wv[:, s * 128 : (s + 1) * 128], 128),
                "Wq8": pack_pj(
                    Wq_p[:, s * 512 : (s + 1) * 512], NH * 128, float8_e4m3
                ),
                "Wk8": pack_pj(Wk_p[:, s * 128 : (s + 1) * 128], 128, float8_e4m3),
                "Wv8": pack_pj(Wv[:, s * 128 : (s + 1) * 128], 128, float8_e4m3),
                "Wo": np.ascontiguousarray(
                    wo_sh.reshape(NH, 128, D).transpose(1, 0, 2)
                ).astype(bfloat16),
                "cosq": cosq,
                "sinq": sinq,
                "cosk": cosk,
                "sink": sink,
                "maskv": maskv,
                "ident": ident,
            }
        )
    return in_maps


def _ensure_ntff_hook():
    """The agent image's antenv lacks axon_hooks, so boot() skips installing
    the NTFF profile hook. Recreate the module and install the hook."""
    import sys
    import types

    try:
        from antenv.axon_hooks import get_axon_ntff_profile_hook  # noqa: F401

        return True
    except ImportError:
        pass
    try:
        import antenv
        from trn_agent_boot.trn_boot import _ntff_profile_via_ctypes

        hook = _ntff_profile_via_ctypes("/opt/axon/libaxon_pjrt.so")
        if hook is None:
            return False
        mod = types.ModuleType("antenv.axon_hooks")
        mod._hook = hook
        mod.set_axon_ntff_profile_hook = lambda h: setattr(mod, "_hook", h)
        mod.get_axon_ntff_profile_hook = lambda: mod._hook
        sys.modules["antenv.axon_hooks"] = mod
        antenv.axon_hooks = mod
        bass_utils.upload_artifacts = lambda d: d
        return True
    except Exception:
        return False


def kernel(x, Wq, Wk, Wv, Wo):
    global _last_results, last_exec_time_ns
    nc = _build_program()
    in_maps = _host_prep(x, Wq, Wk, Wv, Wo)
    trace = bool(int(os.environ.get("KERNEL_TRACE", "0")))
    tmpdir = None
    if trace:
        trace = _ensure_ntff_hook()
        if trace:
            tmpdir = os.environ.get("KERNEL_TRACE_DIR") or None
    res = bass_utils.run_bass_kernel_spmd(
        nc, in_maps, core_ids=list(range(8)), trace=trace, tmpdir=tmpdir
    )
    _last_results = res
    last_exec_time_ns = res.exec_time_ns
    out = np.empty((B, T, D), dtype=np.float32)
    for b in range(B):
        out[b] = sum(
            res.results[4 * b + s]["y"].astype(np.float32) for s in range(TP)
        )
    return out



# revision 69
# speedup vs baseline: 1.0378x; 1.0378x over previous
"""GQA attention (B=2,T=2048,D=2048,H=16,KV=4,HD=128, causal+RoPE) on 8 trn2 cores.

Sharding: 4-way head tensor-parallel x 2-way batch data-parallel.
Core c: batch b=c//4, TP shard s=c%4 -> q heads [4s..4s+3], kv head s.

Transpose-free design (v1): scores are computed directly in kv-major layout
per 128-token kv block:  ST[kv,q] = kT_block^T @ qT_stripe  (PE), so
exp(ST) written to SBUF *is* the P^T operand needed by the PV matmul
O^T[hd,q] = V_block^T @ P^T.  The softmax denominator l[q] = colsum(P^T)
comes from a ones-vector matmul ([128,1] lhsT), its reciprocal is
broadcast to all partitions with a K=1 outer-product matmul, and the
normalization is fused into the PSUM->SBUF move of O^T on the DVE.
Causal masking is multiplicative post-exp (binary bf16 tiles) on the 4
diagonal blocks of each 512-wide q stripe.  No PE transposes anywhere,
so the PE stream is dense back-to-back matmuls and the HAM clock gate
stays at 2.4 GHz.
"""

import math
import os
import numpy as np

try:
    import concourse.bass as bass
except ImportError:  # pragma: no cover
    import sys

    sys.path.insert(0, "/opt/trn_rl_repo")
    import concourse.bass as bass

import concourse.mybir as mybir
import concourse.bacc as bacc
from concourse import bass_utils
from concourse.tile import TileContext
from contextlib import ExitStack
from ml_dtypes import bfloat16, float8_e4m3

B, T, D = 2, 2048, 2048
H, KV, HD = 16, 4, 128
TP = 4  # head-TP ways
NH = H // TP  # q heads per core = 4
NKB = D // 128  # 16 contraction blocks
NTC = T // 512  # 4 token chunks / q stripes
NTB = T // 128  # 16 token blocks
SCALE = 1.0 / math.sqrt(HD)
F32 = mybir.dt.float32
BF16 = mybir.dt.bfloat16
FP8 = mybir.dt.float8e4
DR = mybir.MatmulPerfMode.DoubleRow
EXP = mybir.ActivationFunctionType.Exp

_program = None
_last_results = None
last_exec_time_ns = None
DEFER = bool(int(os.environ.get("KERNEL_DEFER", "1")))  # filler interleave on/off


def _build_program():
    global _program
    if _program is not None:
        return _program

    nc = bacc.Bacc(
        "TRN2",
        target_bir_lowering=False,
        debug=False,
        enable_asserts=False,
        num_devices=8,
    )
    # host-packed layouts: [128 partitions, ...] with j = D/128 contraction blocks
    xp_d = nc.dram_tensor("xp", [128, NKB, 512], BF16, kind="ExternalInput").ap()
    xp8_d = nc.dram_tensor("xp8", [128, 3, NKB, 512], FP8, kind="ExternalInput").ap()
    wq_d = nc.dram_tensor("Wq", [128, NKB, NH * 128], BF16, kind="ExternalInput").ap()
    wk_d = nc.dram_tensor("Wk", [128, NKB, 128], BF16, kind="ExternalInput").ap()
    wv_d = nc.dram_tensor("Wv", [128, NKB, 128], BF16, kind="ExternalInput").ap()
    wq8_d = nc.dram_tensor("Wq8", [128, NKB, NH * 128], FP8, kind="ExternalInput").ap()
    wk8_d = nc.dram_tensor("Wk8", [128, NKB, 128], FP8, kind="ExternalInput").ap()
    wv8_d = nc.dram_tensor("Wv8", [128, NKB, 128], FP8, kind="ExternalInput").ap()
    wo_d = nc.dram_tensor("Wo", [128, NH, D], BF16, kind="ExternalInput").ap()
    cq_d = nc.dram_tensor("cosq", [128, T], BF16, kind="ExternalInput").ap()
    sq_d = nc.dram_tensor("sinq", [128, T], BF16, kind="ExternalInput").ap()
    ck_d = nc.dram_tensor("cosk", [128, T], BF16, kind="ExternalInput").ap()
    sk_d = nc.dram_tensor("sink", [128, T], BF16, kind="ExternalInput").ap()
    mk_d = nc.dram_tensor("maskv", [128, 4, 512], BF16, kind="ExternalInput").ap()
    id_d = nc.dram_tensor("ident", [128, 128], BF16, kind="ExternalInput").ap()
    y_d = nc.dram_tensor("y", [T, D], BF16, kind="ExternalOutput").ap()

    with TileContext(nc) as tc, ExitStack() as ctx:
        big = ctx.enter_context(tc.tile_pool(name="big", bufs=1))
        xpool = ctx.enter_context(tc.tile_pool(name="xpool", bufs=2))
        ps = ctx.enter_context(tc.tile_pool(name="ps", bufs=2, space="PSUM"))
        ps_l = ctx.enter_context(tc.tile_pool(name="ps_l", bufs=2, space="PSUM"))
        ps_y = ctx.enter_context(tc.tile_pool(name="ps_y", bufs=2, space="PSUM"))
        rtmp = ctx.enter_context(tc.tile_pool(name="rtmp", bufs=3))
        vtpool = ctx.enter_context(tc.tile_pool(name="vtpool", bufs=2))
        ptpool = ctx.enter_context(tc.tile_pool(name="ptpool", bufs=2))
        otpool = ctx.enter_context(tc.tile_pool(name="otpool", bufs=2))
        rlpool = ctx.enter_context(tc.tile_pool(name="rlpool", bufs=2))
        bcpool = ctx.enter_context(tc.tile_pool(name="bcpool", bufs=2))
        ypool = ctx.enter_context(tc.tile_pool(name="ypool", bufs=2))

        wq = big.tile([128, NKB, NH * 128], BF16, tag="wq")  # loaded in 4 pieces
        wk = big.tile([128, NKB, 128], BF16, tag="wk")
        wv = big.tile([128, NKB, 128], BF16, tag="wv")
        wq8 = big.tile([128, NKB, NH * 128], FP8, tag="wq8")
        wk8 = big.tile([128, NKB, 128], FP8, tag="wk8")
        wv8 = big.tile([128, NKB, 128], FP8, tag="wv8")
        wo = big.tile([128, NH, D], BF16, tag="wo")
        cq = big.tile([128, T], BF16, tag="cq")
        sq = big.tile([128, T], BF16, tag="sq")
        ck = big.tile([128, T], BF16, tag="ck")
        sk = big.tile([128, T], BF16, tag="sk")
        maskv = big.tile([128, 4, 512], BF16, tag="maskv")
        ident = big.tile([128, 128], BF16, tag="ident")
        onesc = big.tile([128, 1], BF16, tag="onesc")
        onesr = big.tile([1, 128], BF16, tag="onesr")
        warm = big.tile([128, 512], BF16, tag="warm")
        wsink = big.tile([128, 16], F32, tag="wsink")
        qT = big.tile([128, NH, T], BF16, tag="qT")
        kT = big.tile([128, T], BF16, tag="kT")
        V = big.tile([128, NTB, 128], BF16, tag="V")

        # ---- PE warmup: ~6us of dummy matmuls during the initial DMA wait
        # gets the HAM clock gate to 2.4 GHz before the first real matmul.
        nc.vector.memset(warm[:], 0.125)
        nc.vector.memset(onesc[:], 1.0)
        nc.vector.memset(onesr[:], 1.0)
        wp = ps.tile([128, 2, 512], F32, tag="st", name="pst")[:, 0, :]
        for i in range(4):
            nc.tensor.matmul(
                wp[:], lhsT=warm[:, :128], rhs=warm[:], start=(i == 0), stop=(i == 3)
            )
        nc.vector.tensor_copy(wsink[:], wp[:, :16])

        # ---- loads (ordered so early compute unblocks fast) ----
        xcs = {}

        def fetch_x(c):
            if c == 0:
                t = xpool.tile([128, NKB, 512], BF16, tag="xc", bufs=1)
                nc.sync.dma_start(out=t[:], in_=xp_d[:])
            else:
                t = xpool.tile([128, NKB, 512], FP8, tag="xc8")
                nc.sync.dma_start(out=t[:], in_=xp8_d[:, c - 1])
            xcs[c] = t

        # fp8 essentials first: chunk 1 (fp8) is the first compute and needs
        # only ~2MB to start; the 5MB bf16 set for chunk 0 streams in behind
        # ~22us of fp8 chunk-1/2 compute.
        nc.sync.dma_start(out=wk8[:], in_=wk8_d[:])
        fetch_x(1)
        nc.sync.dma_start(out=wq8[:], in_=wq8_d[:])
        nc.sync.dma_start(out=wv8[:], in_=wv8_d[:])
        nc.sync.dma_start(out=ck[:], in_=ck_d[:])
        nc.sync.dma_start(out=sk[:], in_=sk_d[:])
        nc.sync.dma_start(out=cq[:], in_=cq_d[:])
        nc.sync.dma_start(out=sq[:], in_=sq_d[:])
        nc.sync.dma_start(out=ident[:], in_=id_d[:])
        fetch_x(2)
        nc.sync.dma_start(out=wk[:], in_=wk_d[:])
        fetch_x(0)
        for jp in range(4):
            nc.sync.dma_start(
                out=wq[:, 4 * jp : 4 * (jp + 1), :], in_=wq_d[:, 4 * jp : 4 * (jp + 1), :]
            )
        nc.sync.dma_start(out=wv[:], in_=wv_d[:])
        nc.sync.dma_start(out=maskv[:], in_=mk_d[:])
        nc.sync.dma_start(out=wo[:], in_=wo_d[:])

        # ---- projections with fused RoPE, chunk-major over tokens ----
        # The PSUM->SBUF staging copy (ACT) frees the psum slot right away so
        # a late rope (waiting on cos/sin DMAs) never stalls the PE via pool
        # WAR; rope then runs from SBUF at its leisure.
        def rope(pst, cos_sb, sin_sb, dst, sl, gadd=False):
            # 3-way engine split (ops are free-dim-bound, ~0.45-0.7us per
            # 512-wide op regardless of partitions): ACT lifts the
            # half-swapped copy out of PSUM, DVE does the cos mul (PSUM
            # read) + final add, gpsimd the sin mul (SBUF only).
            pqs = rtmp.tile([128, 512], BF16, tag="pqs", bufs=4)
            nc.scalar.copy(pqs[0:64, :], pst[64:128, :])
            nc.scalar.copy(pqs[64:128, :], pst[0:64, :])
            t1 = rtmp.tile([128, 512], BF16, tag="t1")
            nc.vector.tensor_mul(t1[:], pst[:], cos_sb[:, sl])
            t2 = rtmp.tile([128, 512], BF16, tag="t2")
            nc.gpsimd.tensor_mul(t2[:], pqs[:], sin_sb[:, sl])
            if gadd:
                nc.gpsimd.tensor_add(dst, t1[:], t2[:])
            else:
                nc.vector.tensor_add(dst, t1[:], t2[:])

        # projection matmuls: chunk 0 runs bf16 (16 K=128 matmuls); chunks
        # 1-3 run fp8 DoubleRow (8 K=256 matmuls over adjacent j pairs) --
        # early tokens see little softmax averaging so they keep bf16.
        def proj(pst, wbf, wf8, xc, c, cs=slice(None)):
            if c == 0:
                for j in range(NKB):
                    nc.tensor.matmul(
                        pst[:],
                        lhsT=wbf[:, j, cs],
                        rhs=xc[:, j, :],
                        start=(j == 0),
                        stop=(j == NKB - 1),
                    )
            else:
                for u in range(NKB // 2):
                    nc.tensor.matmul(
                        pst[:],
                        lhsT=wf8[:, 2 * u : 2 * u + 2, cs],
                        rhs=xc[:, 2 * u : 2 * u + 2, :],
                        start=(u == 0),
                        stop=(u == NKB // 2 - 1),
                        perf_mode=DR,
                    )

        # Each chunk's projection work as a list of closures ("units"):
        # chunks 0-2 are emitted inline; chunk 3's units go to the filler
        # queue and are interleaved into stripe-0 attention so the PE never
        # idles on the exp latency there (that idle used to re-throttle HAM).
        def chunk_units(c, fetch_next=None):
            sl = slice(c * 512, (c + 1) * 512)
            xc = xcs.pop(c)
            units = []

            gadd = c == NTC - 1  # filler-chunk rope adds ride on gpsimd

            def kt_unit():
                pst = ps.tile([128, 2, 512], F32, tag="st", name="pst")[:, 0, :]
                proj(pst, wk, wk8, xc, c)
                rope(pst, ck, sk, kT[:, sl], sl, gadd)

            units.append(kt_unit)
            for h in range(NH):

                def qt_unit(h=h):
                    pst = ps.tile([128, 2, 512], F32, tag="st", name="pst")[:, 0, :]
                    proj(pst, wq, wq8, xc, c, slice(h * 128, (h + 1) * 128))
                    rope(pst, cq, sq, qT[:, h, sl], sl, gadd)

                units.append(qt_unit)

            def v_unit():
                # V^T chunk (hd on partitions), then 128x128 PE transposes
                # into the tokens-major V needed by the PV matmul.
                pst = ps.tile([128, 2, 512], F32, tag="st", name="pst")[:, 0, :]
                proj(pst, wv, wv8, xc, c)
                vtc = vtpool.tile([128, 512], BF16, tag="vt")
                nc.scalar.copy(vtc[:], pst[:])
                for tb in range(4):
                    tp = ps.tile([128, 2, 512], BF16, tag="st", name="tp")[:, 0, :128]
                    nc.tensor.transpose(
                        tp[:], vtc[:, tb * 128 : (tb + 1) * 128], ident[:]
                    )
                    nc.vector.tensor_copy(V[:, c * 4 + tb, :], tp[:])
                if fetch_next is not None:
                    fetch_x(fetch_next)

            units.append(v_unit)
            return units

        filler = []

        def feed(n=1):
            for _ in range(n):
                if not filler:
                    return
                filler.pop(0)()

        # chunk order 1, 2, 0, 3: the two fp8 chunks run while chunk 0's
        # bf16 weights/x stream in.  Chunk 3 plus the tail of chunk 2 (qT
        # h2/h3 + V, first needed by stripe 2) fill stripe-0 attention.
        c2_units = chunk_units(2, None)
        for u in chunk_units(1, 3):
            u()
        for u in c2_units[:3]:
            u()
        for u in chunk_units(0, None):
            u()
        if DEFER:
            filler.extend(c2_units[3:])
            filler.extend(chunk_units(NTC - 1))
        else:
            for u in c2_units[3:]:
                u()
            for u in chunk_units(NTC - 1):
                u()

        # ---- attention + output projection, per q stripe ----
        LN = mybir.ActivationFunctionType.Ln
        for s in range(NTC):
            qsl = slice(s * 512, (s + 1) * 512)
            nb = 4 * (s + 1)
            OTs = otpool.tile([128, NH, 512], BF16, tag="OT")

            # finalize head h: reciprocal on the narrow [1,512] l row first,
            # broadcast 1/l across partitions (K=1 outer product on PE),
            # stage to SBUF on ACT, then one DVE mul normalizes O^T.
            def finalize(h, lps, op, ot):
                # reciprocal straight from the [1,512] l-row in PSUM (frees
                # the ps_l slot), tiny bf16 cast, PE broadcast, ACT stage,
                # one DVE mul to normalize O^T.
                rl32 = rlpool.tile([1, 512], F32, tag="rl32")
                nc.vector.reciprocal_approx_fast(rl32[:], lps[:])
                rlb = rlpool.tile([1, 512], BF16, tag="rlb")
                nc.vector.tensor_copy(rlb[:], rl32[:])
                bc = ps.tile([128, 2, 512], F32, tag="st", name="pst")[:, 0, :]
                nc.tensor.matmul(bc[:], lhsT=onesr[:], rhs=rlb[:], start=True, stop=True)
                bcs = bcpool.tile([128, 512], BF16, tag="bcs")
                nc.scalar.copy(bcs[:], bc[:])
                nc.vector.tensor_mul(ot[:, h, :], op[:], bcs[:])

            pending = None
            for h in range(NH):
                PT = ptpool.tile([128, NTB, 512], BF16, tag="PT")
                PTacc = ptpool.tile([128, 2, 512], BF16, tag="PTacc")
                lp = ps_l.tile([1, 512], F32, tag="lp")
                op = ps_y.tile([128, 512], F32, tag="acc", name="op")

                # pair g covering kv blocks (2g, 2g+1); the second diagonal
                # pair only touches q columns [256:512] (the rest is masked),
                # so S/exp/mask run on the narrowed pair window, and PV on a
                # per-block window.  The softmax denominator is a pair-wide
                # bf16 running sum on the DVE, reduced over partitions with
                # two accumulating ones-matmuls per head.
                npairs = nb // 2

                def wlo(g):
                    return 256 if 2 * g == 4 * s + 2 else 0

                def blo(b):
                    r = b - 4 * s
                    return 128 * r if r > 0 else 0

                def psum_acc2(g):
                    # one [128, 2, W] DVE op per kv-block pair; the narrower
                    # block's extra columns are mask-zeroed so they add 0
                    lo = blo(2 * g)
                    if g == 0:
                        nc.vector.tensor_copy(PTacc[:], PT[:, 0:2, :])
                    else:
                        nc.vector.tensor_add(
                            PTacc[:, :, lo:], PTacc[:, :, lo:], PT[:, 2 * g : 2 * g + 2, lo:]
                        )

                def pvmm(b):
                    lo = blo(b)
                    nc.tensor.matmul(
                        op[:, lo:],
                        lhsT=V[:, b, :],
                        rhs=PT[:, b, lo:],
                        start=(b == 0),
                        stop=(b == nb - 1),
                        skip_group_check=True,
                    )

                def lpvq(g0, g1):
                    for g in (g0, g1):
                        pvmm(2 * g)
                        pvmm(2 * g + 1)

                for g in range(npairs):
                    lo = wlo(g)
                    stg = ps.tile([128, 2, 512], F32, tag="st")
                    for u in (0, 1):
                        b = 2 * g + u
                        nc.tensor.matmul(
                            stg[:, u, lo:],
                            lhsT=kT[:, b * 128 : (b + 1) * 128],
                            rhs=qT[:, h, s * 512 + lo : (s + 1) * 512],
                            start=True,
                            stop=True,
                        )
                    nc.scalar.activation(
                        PT[:, 2 * g : 2 * g + 2, lo:], stg[:, :, lo:], EXP
                    )
                    if 2 * g >= 4 * s:
                        # mask rows r, r+1 are all-ones beyond lo+256, so the
                        # multiply only needs the 256-wide diagonal window
                        r = 2 * g - 4 * s
                        nc.vector.tensor_mul(
                            PT[:, 2 * g : 2 * g + 2, lo : lo + 256],
                            PT[:, 2 * g : 2 * g + 2, lo : lo + 256],
                            maskv[:, r : r + 2, lo : lo + 256],
                        )
                    if g >= 3 and g % 2 == 1:
                        lpvq(g - 3, g - 2)
                    # l-accumulation emitted after the PV issue point: the
                    # counter-based DVE semaphores make PV wait on every DVE
                    # op emitted before its mask-mul, so keep PTacc behind it
                    psum_acc2(g)
                    if g == 0 and pending is not None and len(pending) == 4:
                        ph, pacc, plp, pop = pending
                        nc.tensor.matmul(
                            plp[:], lhsT=onesc[:], rhs=pacc[:, 0, :],
                            start=True, stop=False,
                        )
                        nc.tensor.matmul(
                            plp[:], lhsT=onesc[:], rhs=pacc[:, 1, :],
                            start=False, stop=True,
                        )
                        pending = (ph, plp, pop)
                    if g == 1 and pending is not None:
                        finalize(*pending, OTs)
                        pending = None
                    if g >= 2:
                        feed(1)
                feed(3 if s == 0 else 1)
                lpvq(npairs - 2, npairs - 1)
                pending = (h, PTacc, lp, op)

            # Wo: y[q, :] = sum_h O_h[q, :] @ Wo_h, deferred as filler units
            # fed into the NEXT stripe's attention stream (so exp latency and
            # head boundaries there hide behind dense Wo matmuls).  Unit A
            # finalizes the last head (l-matmul, broadcast, normalize); each
            # (qb, dc) unit is a 4-matmul accumulation into a fast-rotating
            # "st"-pool bank -- never ps_y/ps_l, whose slots pace the next
            # stripe's PV/l chain (a WAR wait there would deadlock the PE
            # FIFO against not-yet-emitted instructions).
            def wo_units(s, OTs, pend):
                ph, pacc, plp, pop = pend
                units = []

                def unit_a():
                    nc.tensor.matmul(
                        plp[:], lhsT=onesc[:], rhs=pacc[:, 0, :],
                        start=True, stop=False,
                    )
                    nc.tensor.matmul(
                        plp[:], lhsT=onesc[:], rhs=pacc[:, 1, :],
                        start=False, stop=True,
                    )
                    finalize(ph, plp, pop, OTs)

                units.append(unit_a)
                ysbs = {}

                def unit_qd(qb, dc):
                    def run():
                        if dc == 0:
                            ysbs[qb] = ypool.tile([128, D], BF16, tag="y", name="ysb")
                        ysb = ysbs[qb]
                        yp = ps.tile([128, 2, 512], F32, tag="st", name="yp")[:, 0, :]
                        for h in range(NH):
                            nc.tensor.matmul(
                                yp[:],
                                lhsT=OTs[:, h, qb * 128 : (qb + 1) * 128],
                                rhs=wo[:, h, dc * 512 : (dc + 1) * 512],
                                start=(h == 0),
                                stop=(h == NH - 1),
                                skip_group_check=True,
                            )
                        # engine per destination stripe: Wo(0) runs inside
                        # stripe 1 (DVE-paced) -> all-ACT; Wo(2) runs inside
                        # stripe 3 (exp/ACT-paced) -> all-DVE; others split.
                        if s == 0:
                            dve = False
                        elif s == 2:
                            dve = True
                        else:
                            dve = dc % 2 == 0
                        if dve:
                            nc.vector.tensor_copy(
                                ysb[:, dc * 512 : (dc + 1) * 512], yp[:]
                            )
                        else:
                            nc.scalar.copy(ysb[:, dc * 512 : (dc + 1) * 512], yp[:])
                        if dc == 3:
                            nc.sync.dma_start(
                                out=y_d[
                                    s * 512 + qb * 128 : s * 512 + (qb + 1) * 128, :
                                ],
                                in_=ysb[:],
                            )

                    return run

                for qb in range(4):
                    for dc in range(4):
                        units.append(unit_qd(qb, dc))
                return units

            if DEFER:
                filler.extend(wo_units(s, OTs, pending))
            else:
                for u in wo_units(s, OTs, pending):
                    u()
            pending = None

        while filler:
            feed(1)

    nc.compile()
    _program = nc
    return nc


def _host_prep(x, Wq, Wk, Wv, Wo):
    x = np.asarray(x, dtype=np.float32)
    Wq = np.asarray(Wq, dtype=np.float32)
    Wk = np.asarray(Wk, dtype=np.float32)
    Wv = np.asarray(Wv, dtype=np.float32)
    Wo = np.asarray(Wo, dtype=np.float32)

    # RoPE even/odd gather folded into weight column permutation (per head)
    perm128 = np.r_[np.arange(0, 128, 2), np.arange(1, 128, 2)]
    permq = np.concatenate([hb * 128 + perm128 for hb in range(H)])
    permk = np.concatenate([hb * 128 + perm128 for hb in range(KV)])
    Wq_p = Wq[:, permq]
    Wk_p = Wk[:, permk]

    pos = np.arange(T, dtype=np.float64)
    inv_freq = 1.0 / (10000.0 ** (np.arange(0, HD, 2, dtype=np.float64) / HD))
    ang = np.einsum("t,f->tf", pos, inv_freq)  # [T, 64]
    cos = np.cos(ang).T.astype(np.float32)  # [64, T]
    sin = np.sin(ang).T.astype(np.float32)
    cosk_f = np.concatenate([cos, cos], axis=0)  # [128, T]
    sink_f = np.concatenate([-sin, sin], axis=0)
    cosq = (cosk_f * SCALE).astype(bfloat16)
    sinq = (sink_f * SCALE).astype(bfloat16)
    cosk = cosk_f.astype(bfloat16)
    sink = sink_f.astype(bfloat16)

    # binary causal masks for the 4 diagonal blocks of a 512-wide q stripe:
    # keep (kv_l <= q_l - 128*r) for relative kv block r
    kv_l = np.arange(128)[:, None]
    q_l = np.arange(512)[None, :]
    maskv = np.ascontiguousarray(
        np.stack([(kv_l <= q_l - 128 * r) for r in range(4)], axis=1)
    ).astype(bfloat16)  # [128, 4, 512]
    ident = np.eye(128, dtype=bfloat16)

    def pack_pj(w, cols, dt=bfloat16):
        # [D, cols] -> [128, NKB, cols] with [p, j, c] = w[j*128+p, c]
        return np.ascontiguousarray(
            w.reshape(NKB, 128, cols).transpose(1, 0, 2)
        ).astype(dt)

    in_maps = []
    for c in range(8):
        b, s = c // 4, c % 4
        xb = np.ascontiguousarray(x[b].T)  # [D, T]
        wo_sh = Wo[s * 512 : (s + 1) * 512, :]  # [512, D]
        in_maps.append(
            {
                "xp": pack_pj(xb[:, :512], 512),
                "xp8": np.ascontiguousarray(
                    np.stack(
                        [
                            pack_pj(xb[:, 512 * cc : 512 * (cc + 1)], 512, float8_e4m3)
                            for cc in range(1, 4)
                        ],
                        axis=1,
                    )
                ),
                "Wq": pack_pj(Wq_p[:, s * 512 : (s + 1) * 512], NH * 128),
                "Wk": pack_pj(Wk_p[:, s * 128 : (s + 1) * 128], 128),
                "Wv": pack_pj(Wv[:, s * 128 : (s + 1) * 128], 128),
                "Wq8": pack_pj(
                    Wq_p[:, s * 512 : (s + 1) * 512], NH * 128, float8_e4m3
                ),
                "Wk8": pack_pj(Wk_p[:, s * 128 : (s + 1) * 128], 128, float8_e4m3),
                "Wv8": pack_pj(Wv[:, s * 128 : (s + 1) * 128], 128, float8_e4m3),
                "Wo": np.ascontiguousarray(
                    wo_sh.reshape(NH, 128, D).transpose(1, 0, 2)
                ).astype(bfloat16),
                "cosq": cosq,
                "sinq": sinq,
                "cosk": cosk,
                "sink": sink,
                "maskv": maskv,
                "ident": ident,
            }
        )
    return in_maps


def _ensure_ntff_hook():
    """The agent image's antenv lacks axon_hooks, so boot() skips installing
    the NTFF profile hook. Recreate the module and install the hook."""
    import sys
    import types

    try:
        from antenv.axon_hooks import get_axon_ntff_profile_hook  # noqa: F401

        return True
    except ImportError:
        pass
    try:
        import antenv
        from trn_agent_boot.trn_boot import _ntff_profile_via_ctypes

        hook = _ntff_profile_via_ctypes("/opt/axon/libaxon_pjrt.so")
        if hook is None:
            return False
        mod = types.ModuleType("antenv.axon_hooks")
        mod._hook = hook
        mod.set_axon_ntff_profile_hook = lambda h: setattr(mod, "_hook", h)
        mod.get_axon_ntff_profile_hook = lambda: mod._hook
        sys.modules["antenv.axon_hooks"] = mod
        antenv.axon_hooks = mod
        bass_utils.upload_artifacts = lambda d: d
        return True
    except Exception:
        return False


def kernel(x, Wq, Wk, Wv, Wo):
    global _last_results, last_exec_time_ns
    nc = _build_program()
    in_maps = _host_prep(x, Wq, Wk, Wv, Wo)
    trace = bool(int(os.environ.get("KERNEL_TRACE", "0")))
    tmpdir = None
    if trace:
        trace = _ensure_ntff_hook()
        if trace:
            tmpdir = os.environ.get("KERNEL_TRACE_DIR") or None
    res = bass_utils.run_bass_kernel_spmd(
        nc, in_maps, core_ids=list(range(8)), trace=trace, tmpdir=tmpdir
    )
    _last_results = res
    last_exec_time_ns = res.exec_time_ns
    out = np.empty((B, T, D), dtype=np.float32)
    for b in range(B):
        out[b] = sum(
            res.results[4 * b + s]["y"].astype(np.float32) for s in range(TP)
        )
    return out

